# revision 8
# baseline (speedup 1.0000x reference)
"""Trainium2 Bass kernel: ActionEmbedder (1x1 conv on spatially-tiled action).

y[b,e] = relu(sum_a action[b,a] * conv_w[e,a] + conv_b[e])
out[b,e,h,w] = y[b,e]  (broadcast over 64x64 spatial positions)

Sharding: data-parallel over batch B=128 across 8 cores (16 rows each);
conv_w/conv_b replicated. Each core computes its 16x256 y block with 4
matmuls, then broadcasts it into [16*256, 4096] rows and streams 64 MiB
to HBM — the kernel is HBM-write-bandwidth bound.

Straggler-engine rebalance: traces show that on ~20-40%% of cores one SDMA
engine — always engine 0 or 15 — runs ~22%% slower (191us busy vs 157us
for the same 4 MiB), gating the whole kernel. Full-width HWDGE stores pin
1/16 of the bytes on every engine (descriptor swizzle), and HWDGE partial-
partition stores collapse onto engines 0-3/0-7, so the relief path uses
SWDGE (gpsimd) stores: SWDGE assigns descriptor j (cumulative across all
SWDGE DMAs on the queue) to engine (j mod 16) — measured, see probe2/3.
RELIEF_TILES of the 16 batch tiles are stored via SWDGE with 14-desc DMAs
aligned to lanes 1-14 (plus 2x512B pad descs between them, ~47ns each on
engines 15/0), giving engines 0/15 one 32KB desc per relief tile instead
of eight. Per-engine bytes: E0/E15 3.42 MiB, E1-14 4.19 MiB (ratio 0.816
= measured slow/fast rate 21.9/26.9 GB/s), so a straggling E0/E15 finishes
with the pack instead of 30us late, and healthy cores are unchanged.
"""

import os
import sys

import numpy as np

B, A, E, H, W = 128, 256, 256, 64, 64
NCORES = 8
BC = B // NCORES  # 16 batch rows per core
HW = H * W  # 4096 spatial positions
ROWS = BC * E  # 4096 output rows per core, each HW f32 long
TILE_F = 2 * HW  # fill-tile free dim: one batch row (= 2 e-halves) per tile

RELIEF_TILES = (4, 8, 12)  # tiles stored via SWDGE lane-skewed DMAs


def _ensure_import_path():
    try:
        import concourse.bass  # noqa: F401
    except ImportError:
        for p in ("/opt/trn_rl_repo", os.path.expanduser("~/.axon_site/_ro/trn_rl_repo")):
            if os.path.isdir(p) and p not in sys.path:
                sys.path.insert(0, p)
        import concourse.bass  # noqa: F401


_NC = None


def _build():
    """Build (once) the single-core SPMD Bass program."""
    global _NC
    if _NC is not None:
        return _NC
    _ensure_import_path()
    import concourse.bacc as bacc
    import concourse.mybir as mybir
    import concourse.tile as tile

    fp32 = mybir.dt.float32
    # Bacc (not plain Bass): its compile() runs generate_event_semaphores,
    # which splits multi-wait instructions into EventSemaphore + inst — the
    # TRN2 ISA allows at most one sync wait per regular instruction.
    nc = bacc.Bacc("TRN2", target_bir_lowering=False, debug=False, num_devices=NCORES)

    # All per-core inputs packed into one [128, 546] tensor (single DMA, so
    # downstream matmuls wait on a single DMA semaphore — the PE instruction
    # has very few sync-wait slots). E is permuted even/odd on the host so
    # that partition p ends up holding y[., e=2p+j] for parity j — then each
    # partition's two output rows per batch block (2p, 2p+1) are CONTIGUOUS
    # 32KB in DRAM, halving DMA descriptor count vs the identity layout.
    # Host-side layout along the free dim ((i, j) = (A-chunk, E-parity)):
    #   [(2i+j)*128 : (2i+j+1)*128)  lhsT(i,j)[p, m] = conv_w[2m+j, 128i+p]
    #   [512:528)   actT chunk0 act0[p, b] = action[b, p]
    #   [528:544)   actT chunk1 act1[p, b] = action[b, 128 + p]
    #   [544]       bias_j=0[p] = conv_b[2p]
    #   [545]       bias_j=1[p] = conv_b[2p + 1]
    F_PACKED = 2 * E + 2 * BC + 2
    packed = nc.dram_tensor("packed", [128, F_PACKED], fp32, kind="ExternalInput")
    out = nc.dram_tensor("out", [ROWS, HW], fp32, kind="ExternalOutput")
    # SWDGE pad-descriptor target: 512B writes that only advance the lane
    # counter past engines 15/0. Never read back.
    pad_dst = nc.dram_tensor("padscratch", [2, 4096], fp32, kind="Internal")

    with tile.TileContext(nc) as tc:
        with (
            tc.tile_pool(name="const", bufs=1) as cpool,
            tc.tile_pool(name="psum", bufs=1, space="PSUM") as ppool,
            tc.tile_pool(name="fill", bufs=5) as fpool,
        ):
            # Note on startup: ~7.5us of fixed Tile/NEFF preamble (entry
            # barrier, per-engine register loads, ACT table load) runs before
            # this DMA can even dispatch; warmup DMAs were measured to only
            # delay it. First store lands ~13us in; not further reducible here.
            pk = cpool.tile([128, F_PACKED], fp32, name="pk", tag="pk")
            nc.sync.dma_start(pk[:], packed[:])

            # --- yT[e,b] = relu(w @ action^T + b), e on partitions ---
            # yT columns [j*BC + b] hold y[b, 2p + j] on partition p.
            yT = cpool.tile([128, 2 * BC], fp32, name="yT", tag="yT")
            for j in range(2):  # e-parity
                ps = ppool.tile([128, BC], fp32, name=f"ps{j}", tag=f"ps{j}")
                for i in range(2):  # contraction chunk over A
                    nc.tensor.matmul(
                        ps[:],
                        pk[:, (2 * i + j) * 128 : (2 * i + j + 1) * 128],  # lhsT: [K=a, M]
                        pk[:, 2 * E + i * BC : 2 * E + (i + 1) * BC],  # rhs: [K=a, N=b]
                        start=(i == 0),
                        stop=(i == 1),
                    )
                nc.scalar.activation(
                    yT[:, j * BC : (j + 1) * BC],
                    ps[:],
                    mybir.ActivationFunctionType.Relu,
                    bias=pk[:, 2 * E + 2 * BC + j : 2 * E + 2 * BC + j + 1],
                    scale=1.0,
                )

            # --- broadcast fill + store: tile t = batch row b=t ---
            # Output row r = b*E + e with e = 2p + j: partition p's two rows
            # are adjacent, so it writes one contiguous 32KB run per DMA.
            out_ap = out[:]
            for t in range(BC):
                ft = fpool.tile([128, TILE_F], fp32, name=f"ft{t}", tag="fill")
                base = E * t
                if t < 2:
                    # Startup latency: split the first tile on each ring into
                    # per-parity half-fills + half-DMAs so the first store
                    # dispatches right after relu j=0, without waiting for
                    # relu j=1 and a full 8192-wide fill.
                    rows = out_ap[base : base + E, :].rearrange("(p j) f -> p j f", p=128, j=2)
                    for j in range(2):
                        col = yT[:, j * BC + t : j * BC + t + 1].broadcast_to([128, HW])
                        half = ft[:, j * HW : (j + 1) * HW]
                        if t % 2 == 0:
                            nc.vector.tensor_copy(half, col)
                        else:
                            nc.scalar.activation(half, col, mybir.ActivationFunctionType.Copy)
                        (nc.sync if t % 2 == 0 else nc.scalar).dma_start(rows[:, j, :], half)
                    continue
                # One fused broadcast per tile: cols {t, BC+t} of yT hold
                # y[t, 2p] and y[t, 2p+1]; replicate each across HW.
                cols = yT.rearrange("p (j b) -> p j b", j=2)[:, :, t : t + 1]  # [128,2,1]
                src = cols.broadcast_to([128, 2, HW])
                dst = ft[:].rearrange("p (j f) -> p j f", j=2)
                if t % 2 == 0:
                    nc.vector.tensor_copy(dst, src)
                else:
                    nc.scalar.activation(dst, src, mybir.ActivationFunctionType.Copy)
                dst_ap = out_ap[base : base + E, :].rearrange("(p j) f -> p (j f)", p=128, j=2)
                if t not in RELIEF_TILES:
                    # Alternate HWDGE rings: SP ring for DVE-filled tiles, ACT
                    # ring for ACT-filled tiles (same engine as the fill, so
                    # the dispatch needs no cross-engine semaphore).
                    (nc.sync if t % 2 == 0 else nc.scalar).dma_start(dst_ap, ft[:])
                    continue
                # Relief tile: SWDGE stores, lanes 1-14 carry 14-desc DMAs so
                # engines 0/15 see only one real 32KB desc (partitions 126/127
                # via the lane-15,0 2-desc DMA) plus 8x512B pads each. Lane
                # cursor enters and leaves each tile at 0 (16 pad descs/tile).
                ri = RELIEF_TILES.index(t)
                pc = [0]

                def pad(n, _ri=ri, _pc=pc, _ft=ft):
                    col = 1280 * _ri + 128 * _pc[0]
                    nc.gpsimd.dma_start(
                        pad_dst[0:n, col : col + 128], _ft[0:n, 0:128]
                    )
                    _pc[0] += 1

                pad(1)                                               # lane 0
                nc.gpsimd.dma_start(dst_ap[0:14, :], ft[0:14, :])    # 1-14
                nc.gpsimd.dma_start(dst_ap[126:128, :], ft[126:128, :])  # 15,0
                nc.gpsimd.dma_start(dst_ap[14:28, :], ft[14:28, :])  # 1-14
                for k in range(2, 9):
                    pad(2)                                           # 15,0
                    nc.gpsimd.dma_start(
                        dst_ap[14 * k : 14 * k + 14, :], ft[14 * k : 14 * k + 14, :]
                    )                                                # 1-14
                pad(1)                                               # lane 15

    nc.compile()
    _NC = nc
    return nc


def _in_maps(action, conv_w, conv_b):
    action = np.asarray(action, dtype=np.float32)
    wT = np.asarray(conv_w, dtype=np.float32).T  # [A, E]
    bias = np.asarray(conv_b, dtype=np.float32).reshape(E, 1)
    # lhsT(i,j)[p, m] = conv_w[2m+j, 128i+p] = wT[128i+p, 2m+j]
    w_slices = [wT[128 * i : 128 * (i + 1), j::2] for i in range(2) for j in range(2)]
    parts = [*w_slices, None, None, bias[0::2], bias[1::2]]
    maps = []
    for c in range(NCORES):
        actT = action[c * BC : (c + 1) * BC, :].T  # [A, BC]
        parts[4], parts[5] = actT[:128], actT[128:]
        maps.append({"packed": np.ascontiguousarray(np.concatenate(parts, axis=1))})
    return maps


def _run_spmd(in_maps, **kwargs):
    _ensure_import_path()
    from concourse.bass_utils import run_bass_kernel_spmd

    nc = _build()
    return run_bass_kernel_spmd(nc, in_maps, list(range(NCORES)), **kwargs)


_RUNNER = None


def _make_runner():
    """Persistently-jitted equivalent of bass2jax.run_bass_via_pjrt for this
    kernel (n_cores=8): run_bass_via_pjrt builds a fresh jax.jit per call
    (~25s); caching the jitted shard_map makes repeat kernel() calls fast."""
    global _RUNNER
    if _RUNNER is not None:
        return _RUNNER
    import jax
    from concourse import bass2jax, mybir

    nc = _build()
    bass2jax.install_neuronx_cc_hook()
    partition_name = nc.partition_id_tensor.name if nc.partition_id_tensor else None

    in_names, out_names, out_avals, zero_outs = [], [], [], []
    for alloc in nc.m.functions[0].allocations:
        if not isinstance(alloc, mybir.MemoryLocationSet):
            continue
        name = alloc.memorylocations[0].name
        if alloc.kind == "ExternalInput":
            if name != partition_name:
                in_names.append(name)
        elif alloc.kind == "ExternalOutput":
            shape = tuple(alloc.tensor_shape)
            dtype = mybir.dt.np(alloc.dtype)
            out_names.append(name)
            out_avals.append(jax.core.ShapedArray(shape, dtype))
            zero_outs.append(np.zeros(shape, dtype))
    n_params, n_outs = len(in_names), len(out_avals)
    all_names = in_names + out_names + ([partition_name] if partition_name else [])
    donate = tuple(range(n_params, n_params + n_outs))

    def _body(*args):
        operands = list(args)
        if partition_name is not None:
            operands.append(bass2jax.partition_id_tensor())
        outs = bass2jax._bass_exec_p.bind(
            *operands,
            out_avals=tuple(out_avals),
            in_names=tuple(all_names),
            out_names=tuple(out_names),
            lowering_input_output_aliases=(),
            sim_require_finite=True,
            sim_require_nnan=True,
            nc=nc,
        )
        return tuple(outs)

    devices = jax.devices()[:NCORES]
    mesh = bass2jax.Mesh(np.asarray(devices), ("core",))
    sharded = jax.jit(
        bass2jax.shard_map(
            _body,
            mesh=mesh,
            in_specs=(bass2jax.PartitionSpec("core"),) * (n_params + n_outs),
            out_specs=(bass2jax.PartitionSpec("core"),) * n_outs,
            check_rep=False,
        ),
        donate_argnums=donate,
        keep_unused=True,
    )

    def run(in_maps):
        concat_in = [
            np.concatenate([np.asarray(m[nm]) for m in in_maps], axis=0)
            for nm in in_names
        ]
        concat_zeros = [
            np.zeros((NCORES * z.shape[0], *z.shape[1:]), z.dtype) for z in zero_outs
        ]
        out_arrs = sharded(*concat_in, *concat_zeros)
        return [
            {
                nm: np.asarray(out_arrs[i]).reshape(NCORES, *out_avals[i].shape)[c]
                for i, nm in enumerate(out_names)
            }
            for c in range(NCORES)
        ]

    _RUNNER = run
    return run


def kernel(action, conv_w, conv_b):
    _ensure_import_path()
    results = _make_runner()(_in_maps(action, conv_w, conv_b))
    shards = [results[c]["out"].reshape(BC, E, H, W) for c in range(NCORES)]
    return np.concatenate(shards, axis=0)



# revision 10
# speedup vs baseline: 1.0269x; 1.0269x over previous
"""Trainium2 Bass kernel: ActionEmbedder (1x1 conv on spatially-tiled action).

y[b,e] = relu(sum_a action[b,a] * conv_w[e,a] + conv_b[e])
out[b,e,h,w] = y[b,e]  (broadcast over 64x64 spatial positions)

Sharding: data-parallel over batch B=128 across 8 cores (16 rows each);
conv_w/conv_b replicated. Each core computes its 16x256 y block with 4
matmuls, then broadcasts it into [16*256, 4096] rows and streams 64 MiB
to HBM — the kernel is HBM-write-bandwidth bound.

Straggler-engine rebalance: traces show that on ~20-40%% of cores one SDMA
engine — always engine 0 or 15 — runs ~22%% slower (191us busy vs 157us
for the same 4 MiB), gating the whole kernel. Full-width HWDGE stores pin
1/16 of the bytes on every engine (descriptor swizzle), and HWDGE partial-
partition stores collapse onto engines 0-3/0-7, so the relief path uses
SWDGE (gpsimd) stores: SWDGE assigns descriptor j (cumulative across all
SWDGE DMAs on the queue) to engine (j mod 16) — measured, see probe2/3.
RELIEF_TILES of the 16 batch tiles are stored via SWDGE with 14-desc DMAs
aligned to lanes 1-14 (plus 2x512B pad descs between them, ~47ns each on
engines 15/0), giving engines 0/15 one 32KB desc per relief tile instead
of eight. Per-engine bytes: E0/E15 3.42 MiB, E1-14 4.19 MiB (ratio 0.816
= measured slow/fast rate 21.9/26.9 GB/s), so a straggling E0/E15 finishes
with the pack instead of 30us late, and healthy cores are unchanged.
"""

import os
import sys

import numpy as np

B, A, E, H, W = 128, 256, 256, 64, 64
NCORES = 8
BC = B // NCORES  # 16 batch rows per core
HW = H * W  # 4096 spatial positions
ROWS = BC * E  # 4096 output rows per core, each HW f32 long
TILE_F = 2 * HW  # fill-tile free dim: one batch row (= 2 e-halves) per tile

RELIEF_TILES = (4, 8, 12)  # tiles stored via SWDGE lane-skewed DMAs


def _ensure_import_path():
    try:
        import concourse.bass  # noqa: F401
    except ImportError:
        for p in ("/opt/trn_rl_repo", os.path.expanduser("~/.axon_site/_ro/trn_rl_repo")):
            if os.path.isdir(p) and p not in sys.path:
                sys.path.insert(0, p)
        import concourse.bass  # noqa: F401


_NC = None


def _build():
    """Build (once) the single-core SPMD Bass program."""
    global _NC
    if _NC is not None:
        return _NC
    _ensure_import_path()
    import concourse.bacc as bacc
    import concourse.mybir as mybir
    import concourse.tile as tile

    fp32 = mybir.dt.float32
    # Bacc (not plain Bass): its compile() runs generate_event_semaphores,
    # which splits multi-wait instructions into EventSemaphore + inst — the
    # TRN2 ISA allows at most one sync wait per regular instruction.
    nc = bacc.Bacc("TRN2", target_bir_lowering=False, debug=False, num_devices=NCORES)

    # All per-core inputs packed into one [128, 546] tensor (single DMA, so
    # downstream matmuls wait on a single DMA semaphore — the PE instruction
    # has very few sync-wait slots). E is permuted even/odd on the host so
    # that partition p ends up holding y[., e=2p+j] for parity j — then each
    # partition's two output rows per batch block (2p, 2p+1) are CONTIGUOUS
    # 32KB in DRAM, halving DMA descriptor count vs the identity layout.
    # Host-side layout along the free dim ((i, j) = (A-chunk, E-parity)):
    #   [(2i+j)*128 : (2i+j+1)*128)  lhsT(i,j)[p, m] = conv_w[2m+j, 128i+p]
    #   [512:528)   actT chunk0 act0[p, b] = action[b, p]
    #   [528:544)   actT chunk1 act1[p, b] = action[b, 128 + p]
    #   [544]       bias_j=0[p] = conv_b[2p]
    #   [545]       bias_j=1[p] = conv_b[2p + 1]
    F_PACKED = 2 * E + 2 * BC + 2
    packed = nc.dram_tensor("packed", [128, F_PACKED], fp32, kind="ExternalInput")
    out = nc.dram_tensor("out", [ROWS, HW], fp32, kind="ExternalOutput")
    # SWDGE pad-descriptor target: 512B writes that only advance the lane
    # counter past engines 15/0. Never read back.
    pad_dst = nc.dram_tensor("padscratch", [2, 4096], fp32, kind="Internal")

    with tile.TileContext(nc) as tc:
        with (
            tc.tile_pool(name="const", bufs=1) as cpool,
            tc.tile_pool(name="psum", bufs=1, space="PSUM") as ppool,
            tc.tile_pool(name="fill", bufs=4) as fpool,
            # Relief tiles get their own buffers: SWDGE dispatch is ~0.8us
            # per DMA (21 DMAs/relief tile), so relief stores drain ~25us
            # slower than HWDGE tiles. Sharing the main pool made fills WAR-
            # wait on those slow stores and stalled the whole HWDGE pipeline
            # (measured: fills at 59/104/130/195us instead of every ~9us).
            tc.tile_pool(name="relief", bufs=2) as rfpool,
        ):
            # Note on startup: ~7.5us of fixed Tile/NEFF preamble (entry
            # barrier, per-engine register loads, ACT table load) runs before
            # this DMA can even dispatch; warmup DMAs were measured to only
            # delay it. First store lands ~13us in; not further reducible here.
            pk = cpool.tile([128, F_PACKED], fp32, name="pk", tag="pk")
            nc.sync.dma_start(pk[:], packed[:])

            # --- yT[e,b] = relu(w @ action^T + b), e on partitions ---
            # yT columns [j*BC + b] hold y[b, 2p + j] on partition p.
            yT = cpool.tile([128, 2 * BC], fp32, name="yT", tag="yT")
            for j in range(2):  # e-parity
                ps = ppool.tile([128, BC], fp32, name=f"ps{j}", tag=f"ps{j}")
                for i in range(2):  # contraction chunk over A
                    nc.tensor.matmul(
                        ps[:],
                        pk[:, (2 * i + j) * 128 : (2 * i + j + 1) * 128],  # lhsT: [K=a, M]
                        pk[:, 2 * E + i * BC : 2 * E + (i + 1) * BC],  # rhs: [K=a, N=b]
                        start=(i == 0),
                        stop=(i == 1),
                    )
                nc.scalar.activation(
                    yT[:, j * BC : (j + 1) * BC],
                    ps[:],
                    mybir.ActivationFunctionType.Relu,
                    bias=pk[:, 2 * E + 2 * BC + j : 2 * E + 2 * BC + j + 1],
                    scale=1.0,
                )

            # --- broadcast fill + store: tile t = batch row b=t ---
            # Output row r = b*E + e with e = 2p + j: partition p's two rows
            # are adjacent, so it writes one contiguous 32KB run per DMA.
            out_ap = out[:]
            for t in range(BC):
                pool = rfpool if t in RELIEF_TILES else fpool
                tag = "relief" if t in RELIEF_TILES else "fill"
                ft = pool.tile([128, TILE_F], fp32, name=f"ft{t}", tag=tag)
                base = E * t
                if t < 2:
                    # Startup latency: split the first tile on each ring into
                    # per-parity half-fills + half-DMAs so the first store
                    # dispatches right after relu j=0, without waiting for
                    # relu j=1 and a full 8192-wide fill.
                    rows = out_ap[base : base + E, :].rearrange("(p j) f -> p j f", p=128, j=2)
                    for j in range(2):
                        col = yT[:, j * BC + t : j * BC + t + 1].broadcast_to([128, HW])
                        half = ft[:, j * HW : (j + 1) * HW]
                        if t % 2 == 0:
                            nc.vector.tensor_copy(half, col)
                        else:
                            nc.scalar.activation(half, col, mybir.ActivationFunctionType.Copy)
                        (nc.sync if t % 2 == 0 else nc.scalar).dma_start(rows[:, j, :], half)
                    continue
                # One fused broadcast per tile: cols {t, BC+t} of yT hold
                # y[t, 2p] and y[t, 2p+1]; replicate each across HW.
                cols = yT.rearrange("p (j b) -> p j b", j=2)[:, :, t : t + 1]  # [128,2,1]
                src = cols.broadcast_to([128, 2, HW])
                dst = ft[:].rearrange("p (j f) -> p j f", j=2)
                if t % 2 == 0:
                    nc.vector.tensor_copy(dst, src)
                else:
                    nc.scalar.activation(dst, src, mybir.ActivationFunctionType.Copy)
                dst_ap = out_ap[base : base + E, :].rearrange("(p j) f -> p (j f)", p=128, j=2)
                if t not in RELIEF_TILES:
                    # Alternate HWDGE rings: SP ring for DVE-filled tiles, ACT
                    # ring for ACT-filled tiles (same engine as the fill, so
                    # the dispatch needs no cross-engine semaphore).
                    (nc.sync if t % 2 == 0 else nc.scalar).dma_start(dst_ap, ft[:])
                    continue
                # Relief tile: SWDGE stores, lanes 1-14 carry 14-desc DMAs so
                # engines 0/15 see only one real 32KB desc (partitions 126/127
                # via the lane-15,0 2-desc DMA) plus 8x512B pads each. Lane
                # cursor enters and leaves each tile at 0 (16 pad descs/tile).
                ri = RELIEF_TILES.index(t)
                pc = [0]

                def pad(n, _ri=ri, _pc=pc, _ft=ft):
                    col = 1280 * _ri + 128 * _pc[0]
                    nc.gpsimd.dma_start(
                        pad_dst[0:n, col : col + 128], _ft[0:n, 0:128]
                    )
                    _pc[0] += 1

                pad(1)                                               # lane 0
                nc.gpsimd.dma_start(dst_ap[0:14, :], ft[0:14, :])    # 1-14
                nc.gpsimd.dma_start(dst_ap[126:128, :], ft[126:128, :])  # 15,0
                nc.gpsimd.dma_start(dst_ap[14:28, :], ft[14:28, :])  # 1-14
                for k in range(2, 9):
                    pad(2)                                           # 15,0
                    nc.gpsimd.dma_start(
                        dst_ap[14 * k : 14 * k + 14, :], ft[14 * k : 14 * k + 14, :]
                    )                                                # 1-14
                pad(1)                                               # lane 15

    nc.compile()
    _NC = nc
    return nc


def _in_maps(action, conv_w, conv_b):
    action = np.asarray(action, dtype=np.float32)
    wT = np.asarray(conv_w, dtype=np.float32).T  # [A, E]
    bias = np.asarray(conv_b, dtype=np.float32).reshape(E, 1)
    # lhsT(i,j)[p, m] = conv_w[2m+j, 128i+p] = wT[128i+p, 2m+j]
    w_slices = [wT[128 * i : 128 * (i + 1), j::2] for i in range(2) for j in range(2)]
    parts = [*w_slices, None, None, bias[0::2], bias[1::2]]
    maps = []
    for c in range(NCORES):
        actT = action[c * BC : (c + 1) * BC, :].T  # [A, BC]
        parts[4], parts[5] = actT[:128], actT[128:]
        maps.append({"packed": np.ascontiguousarray(np.concatenate(parts, axis=1))})
    return maps


def _run_spmd(in_maps, **kwargs):
    _ensure_import_path()
    from concourse.bass_utils import run_bass_kernel_spmd

    nc = _build()
    return run_bass_kernel_spmd(nc, in_maps, list(range(NCORES)), **kwargs)


_RUNNER = None


def _make_runner():
    """Persistently-jitted equivalent of bass2jax.run_bass_via_pjrt for this
    kernel (n_cores=8): run_bass_via_pjrt builds a fresh jax.jit per call
    (~25s); caching the jitted shard_map makes repeat kernel() calls fast."""
    global _RUNNER
    if _RUNNER is not None:
        return _RUNNER
    import jax
    from concourse import bass2jax, mybir

    nc = _build()
    bass2jax.install_neuronx_cc_hook()
    partition_name = nc.partition_id_tensor.name if nc.partition_id_tensor else None

    in_names, out_names, out_avals, zero_outs = [], [], [], []
    for alloc in nc.m.functions[0].allocations:
        if not isinstance(alloc, mybir.MemoryLocationSet):
            continue
        name = alloc.memorylocations[0].name
        if alloc.kind == "ExternalInput":
            if name != partition_name:
                in_names.append(name)
        elif alloc.kind == "ExternalOutput":
            shape = tuple(alloc.tensor_shape)
            dtype = mybir.dt.np(alloc.dtype)
            out_names.append(name)
            out_avals.append(jax.core.ShapedArray(shape, dtype))
            zero_outs.append(np.zeros(shape, dtype))
    n_params, n_outs = len(in_names), len(out_avals)
    all_names = in_names + out_names + ([partition_name] if partition_name else [])
    donate = tuple(range(n_params, n_params + n_outs))

    def _body(*args):
        operands = list(args)
        if partition_name is not None:
            operands.append(bass2jax.partition_id_tensor())
        outs = bass2jax._bass_exec_p.bind(
            *operands,
            out_avals=tuple(out_avals),
            in_names=tuple(all_names),
            out_names=tuple(out_names),
            lowering_input_output_aliases=(),
            sim_require_finite=True,
            sim_require_nnan=True,
            nc=nc,
        )
        return tuple(outs)

    devices = jax.devices()[:NCORES]
    mesh = bass2jax.Mesh(np.asarray(devices), ("core",))
    sharded = jax.jit(
        bass2jax.shard_map(
            _body,
            mesh=mesh,
            in_specs=(bass2jax.PartitionSpec("core"),) * (n_params + n_outs),
            out_specs=(bass2jax.PartitionSpec("core"),) * n_outs,
            check_rep=False,
        ),
        donate_argnums=donate,
        keep_unused=True,
    )

    def run(in_maps):
        concat_in = [
            np.concatenate([np.asarray(m[nm]) for m in in_maps], axis=0)
            for nm in in_names
        ]
        concat_zeros = [
            np.zeros((NCORES * z.shape[0], *z.shape[1:]), z.dtype) for z in zero_outs
        ]
        out_arrs = sharded(*concat_in, *concat_zeros)
        return [
            {
                nm: np.asarray(out_arrs[i]).reshape(NCORES, *out_avals[i].shape)[c]
                for i, nm in enumerate(out_names)
            }
            for c in range(NCORES)
        ]

    _RUNNER = run
    return run


def kernel(action, conv_w, conv_b):
    _ensure_import_path()
    results = _make_runner()(_in_maps(action, conv_w, conv_b))
    shards = [results[c]["out"].reshape(BC, E, H, W) for c in range(NCORES)]
    return np.concatenate(shards, axis=0)



# revision 14
# speedup vs baseline: 1.2103x; 1.1786x over previous
"""Trainium2 Bass kernel: ActionEmbedder (1x1 conv on spatially-tiled action).

y[b,e] = relu(sum_a action[b,a] * conv_w[e,a] + conv_b[e])
out[b,e,h,w] = y[b,e]  (broadcast over 64x64 spatial positions)

Sharding: data-parallel over batch B=128 across 8 cores (16 rows each);
conv_w/conv_b replicated. Each core computes its 16x256 y block with 4
matmuls, then broadcasts it into [16*256, 4096] rows and streams 64 MiB
to HBM — the kernel is HBM-write-bandwidth bound.

Straggler-engine rebalance: traces show that on ~20-40%% of cores one SDMA
engine — always engine 0 or 15 — runs ~22%% slower (191us busy vs 157us
for the same 4 MiB), gating the whole kernel. Full-width HWDGE stores pin
1/16 of the bytes on every engine (descriptor swizzle), and HWDGE partial-
partition stores collapse onto engines 0-3/0-7, so the relief path uses
SWDGE (gpsimd) stores: SWDGE assigns descriptor j (cumulative across all
SWDGE DMAs on the queue) to engine (j mod 16) — measured, see probe2/3.
RELIEF_TILES of the 16 batch tiles are stored via SWDGE with 14-desc DMAs
aligned to lanes 1-14 (plus 2x512B pad descs between them, ~47ns each on
engines 15/0), giving engines 0/15 one 32KB desc per relief tile instead
of eight. Per-engine bytes: E0/E15 3.42 MiB, E1-14 4.19 MiB (ratio 0.816
= measured slow/fast rate 21.9/26.9 GB/s), so a straggling E0/E15 finishes
with the pack instead of 30us late, and healthy cores are unchanged.
"""

import os
import sys

import numpy as np

B, A, E, H, W = 128, 256, 256, 64, 64
NCORES = 8
BC = B // NCORES  # 16 batch rows per core
HW = H * W  # 4096 spatial positions
ROWS = BC * E  # 4096 output rows per core, each HW f32 long
TILE_F = 2 * HW  # fill-tile free dim: one batch row (= 2 e-halves) per tile

# Straggler relief via SWDGE lane-skewed stores: DISABLED. The lane rule
# (cumulative desc index mod 16, verified in probe2/probe4) and the pattern
# were correct, but plain gpsimd dma_start on Pool retires only at DMA
# completion (~4-9us per DMA serialized, measured), so the 63 relief DMAs
# trickled until ~245us and regressed every core to ~249us (vs 214.7us
# baseline). A working relief needs the prepare_only+trigger_dma path
# (dma_scatter_add) whose preps pipeline; not landed. HWDGE [0:120) partial
# (engines 0-14, E15 skipped — probe4) is a free E15-relief unit but E0
# stragglers are equally common and HWDGE partials cannot skip E0.
RELIEF_TILES = ()


def _ensure_import_path():
    try:
        import concourse.bass  # noqa: F401
    except ImportError:
        for p in ("/opt/trn_rl_repo", os.path.expanduser("~/.axon_site/_ro/trn_rl_repo")):
            if os.path.isdir(p) and p not in sys.path:
                sys.path.insert(0, p)
        import concourse.bass  # noqa: F401


_NC = None


def _build():
    """Build (once) the single-core SPMD Bass program."""
    global _NC
    if _NC is not None:
        return _NC
    _ensure_import_path()
    import concourse.bacc as bacc
    import concourse.mybir as mybir
    import concourse.tile as tile

    fp32 = mybir.dt.float32
    # Bacc (not plain Bass): its compile() runs generate_event_semaphores,
    # which splits multi-wait instructions into EventSemaphore + inst — the
    # TRN2 ISA allows at most one sync wait per regular instruction.
    nc = bacc.Bacc("TRN2", target_bir_lowering=False, debug=False, num_devices=NCORES)

    # All per-core inputs packed into one [128, 546] tensor (single DMA, so
    # downstream matmuls wait on a single DMA semaphore — the PE instruction
    # has very few sync-wait slots). E is permuted even/odd on the host so
    # that partition p ends up holding y[., e=2p+j] for parity j — then each
    # partition's two output rows per batch block (2p, 2p+1) are CONTIGUOUS
    # 32KB in DRAM, halving DMA descriptor count vs the identity layout.
    # Host-side layout along the free dim ((i, j) = (A-chunk, E-parity)):
    #   [(2i+j)*128 : (2i+j+1)*128)  lhsT(i,j)[p, m] = conv_w[2m+j, 128i+p]
    #   [512:528)   actT chunk0 act0[p, b] = action[b, p]
    #   [528:544)   actT chunk1 act1[p, b] = action[b, 128 + p]
    #   [544]       bias_j=0[p] = conv_b[2p]
    #   [545]       bias_j=1[p] = conv_b[2p + 1]
    F_PACKED = 2 * E + 2 * BC + 2
    packed = nc.dram_tensor("packed", [128, F_PACKED], fp32, kind="ExternalInput")
    out = nc.dram_tensor("out", [ROWS, HW], fp32, kind="ExternalOutput")

    with tile.TileContext(nc) as tc:
        with (
            tc.tile_pool(name="const", bufs=1) as cpool,
            tc.tile_pool(name="psum", bufs=1, space="PSUM") as ppool,
            tc.tile_pool(name="fill", bufs=5) as fpool,
        ):
            # Note on startup: ~7.5us of fixed Tile/NEFF preamble (entry
            # barrier, per-engine register loads, ACT table load) runs before
            # this DMA can even dispatch; warmup DMAs were measured to only
            # delay it. First store lands ~13us in; not further reducible here.
            pk = cpool.tile([128, F_PACKED], fp32, name="pk", tag="pk")
            nc.sync.dma_start(pk[:], packed[:])

            # --- yT[e,b] = relu(w @ action^T + b), e on partitions ---
            # yT columns [j*BC + b] hold y[b, 2p + j] on partition p.
            yT = cpool.tile([128, 2 * BC], fp32, name="yT", tag="yT")
            for j in range(2):  # e-parity
                ps = ppool.tile([128, BC], fp32, name=f"ps{j}", tag=f"ps{j}")
                for i in range(2):  # contraction chunk over A
                    nc.tensor.matmul(
                        ps[:],
                        pk[:, (2 * i + j) * 128 : (2 * i + j + 1) * 128],  # lhsT: [K=a, M]
                        pk[:, 2 * E + i * BC : 2 * E + (i + 1) * BC],  # rhs: [K=a, N=b]
                        start=(i == 0),
                        stop=(i == 1),
                    )
                nc.scalar.activation(
                    yT[:, j * BC : (j + 1) * BC],
                    ps[:],
                    mybir.ActivationFunctionType.Relu,
                    bias=pk[:, 2 * E + 2 * BC + j : 2 * E + 2 * BC + j + 1],
                    scale=1.0,
                )

            # --- broadcast fill + store: tile t = batch row b=t ---
            # Output row r = b*E + e with e = 2p + j: partition p's two rows
            # are adjacent, so it writes one contiguous 32KB run per DMA.
            out_ap = out[:]
            for t in range(BC):
                ft = fpool.tile([128, TILE_F], fp32, name=f"ft{t}", tag="fill")
                base = E * t
                if t < 2:
                    # Startup latency: split the first tile on each ring into
                    # per-parity half-fills + half-DMAs so the first store
                    # dispatches right after relu j=0, without waiting for
                    # relu j=1 and a full 8192-wide fill.
                    rows = out_ap[base : base + E, :].rearrange("(p j) f -> p j f", p=128, j=2)
                    for j in range(2):
                        col = yT[:, j * BC + t : j * BC + t + 1].broadcast_to([128, HW])
                        half = ft[:, j * HW : (j + 1) * HW]
                        if t % 2 == 0:
                            nc.vector.tensor_copy(half, col)
                        else:
                            nc.scalar.activation(half, col, mybir.ActivationFunctionType.Copy)
                        (nc.sync if t % 2 == 0 else nc.scalar).dma_start(rows[:, j, :], half)
                    continue
                # One fused broadcast per tile: cols {t, BC+t} of yT hold
                # y[t, 2p] and y[t, 2p+1]; replicate each across HW.
                cols = yT.rearrange("p (j b) -> p j b", j=2)[:, :, t : t + 1]  # [128,2,1]
                src = cols.broadcast_to([128, 2, HW])
                dst = ft[:].rearrange("p (j f) -> p j f", j=2)
                if t % 2 == 0:
                    nc.vector.tensor_copy(dst, src)
                else:
                    nc.scalar.activation(dst, src, mybir.ActivationFunctionType.Copy)
                dst_ap = out_ap[base : base + E, :].rearrange("(p j) f -> p (j f)", p=128, j=2)
                if t not in RELIEF_TILES:
                    # Alternate HWDGE rings: SP ring for DVE-filled tiles, ACT
                    # ring for ACT-filled tiles (same engine as the fill, so
                    # the dispatch needs no cross-engine semaphore).
                    (nc.sync if t % 2 == 0 else nc.scalar).dma_start(dst_ap, ft[:])
                    continue
                # Relief tile: SWDGE stores, lanes 1-14 carry 14-desc DMAs so
                # engines 0/15 see only one real 32KB desc (partitions 126/127
                # via the lane-15,0 2-desc DMA) plus 8x512B pads each. Lane
                # cursor enters and leaves each tile at 0 (16 pad descs/tile).
                ri = RELIEF_TILES.index(t)
                pc = [0]

                def pad(n, _ri=ri, _pc=pc, _ft=ft):
                    col = 1280 * _ri + 128 * _pc[0]
                    nc.gpsimd.dma_start(
                        pad_dst[0:n, col : col + 128], _ft[0:n, 0:128]
                    )
                    _pc[0] += 1

                pad(1)                                               # lane 0
                nc.gpsimd.dma_start(dst_ap[0:14, :], ft[0:14, :])    # 1-14
                nc.gpsimd.dma_start(dst_ap[126:128, :], ft[126:128, :])  # 15,0
                nc.gpsimd.dma_start(dst_ap[14:28, :], ft[14:28, :])  # 1-14
                for k in range(2, 9):
                    pad(2)                                           # 15,0
                    nc.gpsimd.dma_start(
                        dst_ap[14 * k : 14 * k + 14, :], ft[14 * k : 14 * k + 14, :]
                    )                                                # 1-14
                pad(1)                                               # lane 15

    nc.compile()
    _NC = nc
    return nc


def _in_maps(action, conv_w, conv_b):
    action = np.asarray(action, dtype=np.float32)
    wT = np.asarray(conv_w, dtype=np.float32).T  # [A, E]
    bias = np.asarray(conv_b, dtype=np.float32).reshape(E, 1)
    # lhsT(i,j)[p, m] = conv_w[2m+j, 128i+p] = wT[128i+p, 2m+j]
    w_slices = [wT[128 * i : 128 * (i + 1), j::2] for i in range(2) for j in range(2)]
    parts = [*w_slices, None, None, bias[0::2], bias[1::2]]
    maps = []
    for c in range(NCORES):
        actT = action[c * BC : (c + 1) * BC, :].T  # [A, BC]
        parts[4], parts[5] = actT[:128], actT[128:]
        maps.append({"packed": np.ascontiguousarray(np.concatenate(parts, axis=1))})
    return maps


def _run_spmd(in_maps, **kwargs):
    _ensure_import_path()
    from concourse.bass_utils import run_bass_kernel_spmd

    nc = _build()
    return run_bass_kernel_spmd(nc, in_maps, list(range(NCORES)), **kwargs)


_RUNNER = None


def _make_runner():
    """Persistently-jitted equivalent of bass2jax.run_bass_via_pjrt for this
    kernel (n_cores=8): run_bass_via_pjrt builds a fresh jax.jit per call
    (~25s); caching the jitted shard_map makes repeat kernel() calls fast."""
    global _RUNNER
    if _RUNNER is not None:
        return _RUNNER
    import jax
    from concourse import bass2jax, mybir

    nc = _build()
    bass2jax.install_neuronx_cc_hook()
    partition_name = nc.partition_id_tensor.name if nc.partition_id_tensor else None

    in_names, out_names, out_avals, zero_outs = [], [], [], []
    for alloc in nc.m.functions[0].allocations:
        if not isinstance(alloc, mybir.MemoryLocationSet):
            continue
        name = alloc.memorylocations[0].name
        if alloc.kind == "ExternalInput":
            if name != partition_name:
                in_names.append(name)
        elif alloc.kind == "ExternalOutput":
            shape = tuple(alloc.tensor_shape)
            dtype = mybir.dt.np(alloc.dtype)
            out_names.append(name)
            out_avals.append(jax.core.ShapedArray(shape, dtype))
            zero_outs.append(np.zeros(shape, dtype))
    n_params, n_outs = len(in_names), len(out_avals)
    all_names = in_names + out_names + ([partition_name] if partition_name else [])
    donate = tuple(range(n_params, n_params + n_outs))

    def _body(*args):
        operands = list(args)
        if partition_name is not None:
            operands.append(bass2jax.partition_id_tensor())
        outs = bass2jax._bass_exec_p.bind(
            *operands,
            out_avals=tuple(out_avals),
            in_names=tuple(all_names),
            out_names=tuple(out_names),
            lowering_input_output_aliases=(),
            sim_require_finite=True,
            sim_require_nnan=True,
            nc=nc,
        )
        return tuple(outs)

    devices = jax.devices()[:NCORES]
    mesh = bass2jax.Mesh(np.asarray(devices), ("core",))
    sharded = jax.jit(
        bass2jax.shard_map(
            _body,
            mesh=mesh,
            in_specs=(bass2jax.PartitionSpec("core"),) * (n_params + n_outs),
            out_specs=(bass2jax.PartitionSpec("core"),) * n_outs,
            check_rep=False,
        ),
        donate_argnums=donate,
        keep_unused=True,
    )

    def run(in_maps):
        concat_in = [
            np.concatenate([np.asarray(m[nm]) for m in in_maps], axis=0)
            for nm in in_names
        ]
        concat_zeros = [
            np.zeros((NCORES * z.shape[0], *z.shape[1:]), z.dtype) for z in zero_outs
        ]
        out_arrs = sharded(*concat_in, *concat_zeros)
        return [
            {
                nm: np.asarray(out_arrs[i]).reshape(NCORES, *out_avals[i].shape)[c]
                for i, nm in enumerate(out_names)
            }
            for c in range(NCORES)
        ]

    _RUNNER = run
    return run


def kernel(action, conv_w, conv_b):
    _ensure_import_path()
    results = _make_runner()(_in_maps(action, conv_w, conv_b))
    shards = [results[c]["out"].reshape(BC, E, H, W) for c in range(NCORES)]
    return np.concatenate(shards, axis=0)

